# revision 35
# baseline (speedup 1.0000x reference)
"""Trainium2 Bass kernel for AttentionPair pooling.

Computation (per batch row b):
    t1 = vector @ W_vec                        [B, A]
    t2 = matrix @ W_mat                        [B, S, A]
    t3 = relu(t1[:, None, :] + t2)             [B, S, A]
    logits = t3 @ w_attn                       [B, S]
    attn = masked softmax over valid s         [B, S]
    reps = einsum('bsd,bs->bd', matrix, attn)  [B, D]

Sharding: data-parallel over batch across 8 NeuronCores (8 batches per
core); the three weight tensors are replicated.

Length specialization: positions s >= input_lengths[b] contribute
nothing (attn is exactly 0 there), so per batch only
ceil(len / 128) s-tiles of work are needed.  Batches are sorted by
length and dealt round-robin to the 8 cores, so the shared SPMD
program's slot j only needs tiles for the max length in that slot.
The program is built (and cached) per slot-tile-count tuple at call
time from the actual input_lengths.

Per-core device program (fully unrolled, Tile framework):
  - matrix rows stream in natural [s, d] layout, one batch at a time
  - each batch tile is transposed on the PE (128x128 fp32r transposes)
    into [d, s] layout for the t2 contraction over d
  - t2^T[a, s] accumulates in PSUM over 8 d-tiles (fp32r matmuls)
  - PSUM evacuation is fused with the +t1 bias and relu on the Scalar
    engine; transpose evacuations alternate DVE/ACT
  - logits via w_attn as a [128, 1] stationary operand, per-row masked
    softmax on partition 0 reading logits straight from PSUM
  - attn-weighted sum (reps) is software-pipelined one batch behind so
    the in-order PE queue never stalls on a softmax chain

The global max subtraction in the reference cancels in the per-row
normalization, so a per-row max is used instead (mathematically
identical, numerically safe).
"""

import numpy as np

import concourse.tile as tile
from concourse import bacc, mybir
from concourse.bass_utils import run_bass_kernel_spmd

B, S, DV, DA, DM = 64, 512, 1024, 512, 1024
N_CORES = 8
BPC = B // N_CORES  # batches per core

F32 = mybir.dt.float32
MM_DT = mybir.dt.float32r  # PE runs fp32 data at bf16 rate in this mode

DT = DM // 128   # 8 d-tiles
AT = DA // 128   # 4 a-tiles
ST = S // 128    # 4 s-tiles (max)


def _r(ap):
    return ap.bitcast(MM_DT)


def build_program(st_slots):
    """st_slots[j] = number of 128-row s-tiles to process for batch slot j
    (identical across cores; each >= 2 so fp32r matmuls keep full rate)."""
    nc = bacc.Bacc("TRN2", target_bir_lowering=False, debug=False,
                   num_devices=N_CORES)

    # fp32r-typed DRAM inputs: the PE consumes them in fp32r mode, plain
    # HWDGE DMA (no SWDGE cast pass), bits are the raw fp32 values.
    mat_d = nc.dram_tensor("mat", [BPC, S, DM], MM_DT, kind="ExternalInput")
    wv_d = nc.dram_tensor("wv", [DV, DA], MM_DT, kind="ExternalInput")
    wm_d = nc.dram_tensor("wm", [DM, DA], MM_DT, kind="ExternalInput")
    # packed [ident(128) | waT(AT) | vecT(DT*BPC)] columns, one DMA
    CW = 128 + AT + DT * BPC
    consts_d = nc.dram_tensor("consts", [128, CW], MM_DT, kind="ExternalInput")
    lens_d = nc.dram_tensor("lens", [1, BPC], F32, kind="ExternalInput")

    reps_d = nc.dram_tensor("reps", [BPC, DM], F32, kind="ExternalOutput")
    attn_d = nc.dram_tensor("attn", [BPC, S], F32, kind="ExternalOutput")

    with tile.TileContext(nc) as tc:
        with (
            tc.tile_pool(name="const", bufs=1) as const,
            tc.tile_pool(name="mat_nat", bufs=4) as mat_pool,
            tc.tile_pool(name="matT", bufs=2) as matT_pool,
            tc.tile_pool(name="t3", bufs=2) as t3_pool,
            tc.tile_pool(name="rows", bufs=2) as rows,
            tc.tile_pool(name="ps_tr", bufs=2, space="PSUM") as ps_tr,
            tc.tile_pool(name="ps_mm", bufs=2, space="PSUM") as ps_mm,
            tc.tile_pool(name="ps_lg", bufs=1, space="PSUM") as ps_lg,
            tc.tile_pool(name="ps_at", bufs=1, space="PSUM") as ps_at,
            tc.tile_pool(name="ps_rep", bufs=2, space="PSUM") as ps_rep,
        ):
            # ---- matrix loads + packed constants on the sync ring (no
            # ---- data-dependent waits ever sit ahead of these triggers);
            # ---- weights on the scalar ring run concurrently; outputs go
            # ---- on the scalar ring so their waits can't block loads.
            # ---- The first (shortest) slot's matrix is triggered first so
            # ---- the PE starts as early as possible.
            mat_tiles = {}

            def load_mat(j):
                t = mat_pool.tile([128, ST, DM], MM_DT, tag="mat_nat")
                stj = st_slots[j]
                nc.sync.dma_start(
                    t[:, 0:stj, :],
                    mat_d.ap()[j:j + 1, 0:128 * stj, :].rearrange(
                        "o (t p) d -> p (o t) d", p=128),
                )
                mat_tiles[j] = t

            load_mat(0)

            consts_sb = const.tile([128, CW], MM_DT)
            nc.sync.dma_start(consts_sb[:], consts_d.ap()[:])
            ident = consts_sb[:, 0:128]
            waT_sb = consts_sb[:, 128:128 + AT]
            vecT_sb = consts_sb[:, 128 + AT:].rearrange("p (t b) -> p t b", b=BPC)
            lens_sb = const.tile([1, BPC], F32)
            nc.sync.dma_start(lens_sb[:], lens_d.ap()[:])

            wm_sb = const.tile([128, DT, DA], MM_DT)
            for h in range(2):
                nc.scalar.dma_start(
                    wm_sb[:, 4 * h:4 * (h + 1), :],
                    wm_d.ap()[512 * h:512 * (h + 1), :].rearrange(
                        "(t p) a -> p t a", p=128))
            wv_sb = const.tile([128, DT, DA], MM_DT)
            nc.scalar.dma_start(wv_sb[:], wv_d.ap().rearrange("(t p) a -> p t a", p=128))

            iota_i = const.tile([1, S], mybir.dt.int32)
            nc.gpsimd.iota(iota_i[:], pattern=[[1, S]], base=0, channel_multiplier=0)
            iota_f = const.tile([1, S], F32)
            nc.vector.tensor_copy(iota_f[:], iota_i[:])

            t1T_sb = const.tile([128, AT, BPC], F32)

            def emit_t1():
                # t1 = vector @ W_vec: wide matmuls into [b, a] layout, then
                # PE-transpose the [8, 512] result into per-partition [a, b]
                ps_t1 = ps_lg.tile([BPC, DA], F32, tag="ps_lg")
                for dt in range(DT):
                    nc.tensor.matmul(
                        ps_t1[:],
                        vecT_sb[:, dt, :],
                        wv_sb[:, dt, :],
                        start=(dt == 0), stop=(dt == DT - 1),
                    )
                t1_ba = const.tile([BPC, DA], MM_DT)
                nc.vector.tensor_copy(t1_ba[:], _r(ps_t1[:]))
                ps_tt = ps_tr.tile([128, AT, BPC], F32, tag="ps_tr")
                for at in range(AT):
                    nc.tensor.matmul(
                        _r(ps_tt[:, at, :]),
                        t1_ba[:, at * 128:(at + 1) * 128],
                        ident[0:BPC, 0:BPC],
                        is_transpose=True,
                    )
                nc.vector.tensor_copy(t1T_sb[:], ps_tt[:])

            def emit_reps(attn_row, mat_nat, b, stb):
                # attn^T as [s % 128, s // 128] columns via tiny PE
                # transposes.  fp32r matmuls need even element counts and
                # 8B-aligned PSUM destinations, so transpose two rows at a
                # time (row 1 is junk and ignored downstream).
                ps_a = ps_at.tile([128, 2 * ST], F32, tag="ps_at")
                for st in range(stb):
                    nc.tensor.matmul(
                        _r(ps_a[:, 2 * st:2 * st + 2]),
                        attn_row[0:2, st * 128:(st + 1) * 128],
                        ident[0:2, 0:2],
                        is_transpose=True,
                    )
                attnT = rows.tile([128, 2 * ST], MM_DT)
                nc.vector.tensor_copy(attnT[:, 0:2 * stb], _r(ps_a[:, 0:2 * stb]))

                # reps[d] = sum_s attn[s] * mat[s, d]
                reps_row = rows.tile([1, DM], F32)
                for nh in range(2):
                    ps_r = ps_rep.tile([1, DA], F32, tag="ps_rep")
                    for st in range(stb):
                        nc.tensor.matmul(
                            ps_r[:],
                            attnT[:, 2 * st:2 * st + 1],
                            mat_nat[:, st, nh * DA:(nh + 1) * DA],
                            start=(st == 0), stop=(st == stb - 1),
                        )
                    nc.scalar.copy(reps_row[0:1, nh * DA:(nh + 1) * DA], ps_r[:])
                nc.scalar.dma_start(reps_d.ap()[b:b + 1, :], reps_row[0:1, :])

            def emit_logits_softmax(t3_sb, b, stb):
                wb = 128 * stb
                # logits[s] = sum_a t3[a, s] * w_attn[a]
                ps_l = ps_lg.tile([1, S], F32, tag="ps_lg")
                for at in range(AT):
                    nc.tensor.matmul(
                        ps_l[:, 0:wb],
                        waT_sb[:, at:at + 1],
                        t3_sb[:, at, 0:wb],
                        start=(at == 0), stop=(at == AT - 1),
                    )
                return ps_l

            def emit_softmax(ps_l, b, stb):
                wb = 128 * stb
                # masked softmax on partition 0, reading logits from PSUM
                neg_mx = rows.tile([1, 1], F32)
                nc.vector.tensor_reduce(
                    out=neg_mx[:], in_=ps_l[0:1, 0:wb],
                    op=mybir.AluOpType.max, axis=mybir.AxisListType.X,
                    negate=True,
                )
                exp_row = rows.tile([1, S], F32)
                nc.scalar.activation(
                    exp_row[0:1, 0:wb], ps_l[0:1, 0:wb],
                    mybir.ActivationFunctionType.Exp,
                    bias=neg_mx[0:1, 0:1],
                )
                mask_row = rows.tile([1, S], F32)
                nc.vector.tensor_scalar(
                    out=mask_row[0:1, 0:wb], in0=iota_f[0:1, 0:wb],
                    scalar1=lens_sb[0:1, b:b + 1], scalar2=None,
                    op0=mybir.AluOpType.is_lt,
                )
                masked = rows.tile([1, S], F32)
                nc.vector.tensor_tensor(
                    out=masked[0:1, 0:wb], in0=exp_row[0:1, 0:wb],
                    in1=mask_row[0:1, 0:wb],
                    op=mybir.AluOpType.mult,
                )
                ssum = rows.tile([1, 1], F32)
                nc.vector.tensor_reduce(
                    out=ssum[:], in_=masked[0:1, 0:wb],
                    op=mybir.AluOpType.add, axis=mybir.AxisListType.X,
                )
                rcp = rows.tile([1, 1], F32)
                nc.vector.reciprocal(rcp[:], ssum[:])
                attn_row = rows.tile([2, S], MM_DT)
                nc.gpsimd.memset(attn_row[0:2, :].bitcast(F32), 0.0)
                nc.vector.tensor_scalar_mul(
                    attn_row[0:1, 0:wb], masked[0:1, 0:wb], rcp[0:1, 0:1])
                nc.scalar.dma_start(attn_d.ap()[b:b + 1, :], attn_row[0:1, :].bitcast(F32))
                return attn_row

            # Two-stage software pipeline: logits+softmax for batch b-1 and
            # the attn-weighted sum for batch b-2 are emitted while batch
            # b's transposes/t2 keep the PE dense, so the in-order PE queue
            # never waits on the relu or softmax chains.
            pend_sm = None    # (t3_sb, b, stb) awaiting logits + softmax
            pend_reps = None  # (attn_row, mat_nat, b, stb)

            for b in range(BPC):
                stb = st_slots[b]
                wb = 128 * stb  # valid s width this slot

                if b + 1 < BPC:
                    load_mat(b + 1)
                mat_nat = mat_tiles.pop(b)

                # transpose to [d % 128, d // 128, s] via PE; evacuations
                # alternate between DVE and ACT so neither engine gates
                matT = matT_pool.tile([128, DT, S], MM_DT)
                for dt in range(DT):
                    ps = ps_tr.tile([128, S], F32, tag="ps_tr")
                    for st in range(stb):
                        nc.tensor.matmul(
                            _r(ps[:, st * 128:(st + 1) * 128]),
                            mat_nat[:, st, dt * 128:(dt + 1) * 128],
                            ident[:],
                            is_transpose=True,
                        )
                    if dt % 2 == 0 or b < 2:
                        nc.vector.tensor_copy(matT[:, dt, 0:wb], _r(ps[:, 0:wb]))
                    else:
                        nc.scalar.copy(matT[:, dt, 0:wb], _r(ps[:, 0:wb]))

                # t2^T[a, s] + t1 bias + relu -> t3 [a % 128, a // 128, s]
                t3_sb = t3_pool.tile([128, AT, S], MM_DT)

                def t2_mms(at):
                    ps = ps_mm.tile([128, S], F32, tag="ps_mm")
                    for dt in range(DT):
                        nc.tensor.matmul(
                            ps[:, 0:wb],
                            wm_sb[:, dt, at * 128:(at + 1) * 128],
                            matT[:, dt, 0:wb],
                            start=(dt == 0), stop=(dt == DT - 1),
                        )
                    return ps

                def t2_relu(at, ps):
                    nc.scalar.activation(
                        t3_sb[:, at, 0:wb], ps[:, 0:wb],
                        mybir.ActivationFunctionType.Relu,
                        bias=t1T_sb[:, at, b:b + 1],
                    )

                if b == 0:
                    # t1 slots in between: W_vec arrives after W_mat, and
                    # the relus (which wait on t1T) must be traced after it
                    ps0, ps1 = t2_mms(0), t2_mms(1)
                    emit_t1()
                    t2_relu(0, ps0)
                    t2_relu(1, ps1)
                    for at in range(2, AT):
                        t2_relu(at, t2_mms(at))
                else:
                    for at in range(AT):
                        t2_relu(at, t2_mms(at))

                if pend_sm is not None:
                    p_t3, p_b, p_stb, p_mat = pend_sm
                    ps_l = emit_logits_softmax(p_t3, p_b, p_stb)
                    if pend_reps is not None:
                        emit_reps(*pend_reps)
                    attn_row = emit_softmax(ps_l, p_b, p_stb)
                    pend_reps = (attn_row, p_mat, p_b, p_stb)
                pend_sm = (t3_sb, b, stb, mat_nat)

            # drain the pipeline
            p_t3, p_b, p_stb, p_mat = pend_sm
            ps_l = emit_logits_softmax(p_t3, p_b, p_stb)
            if pend_reps is not None:
                emit_reps(*pend_reps)
            attn_row = emit_softmax(ps_l, p_b, p_stb)
            emit_reps(attn_row, p_mat, p_b, p_stb)

    nc.compile()
    return nc


_PROGRAM_CACHE = {}


def _get_program(st_slots):
    key = tuple(st_slots)
    if key not in _PROGRAM_CACHE:
        _PROGRAM_CACHE[key] = build_program(key)
    return _PROGRAM_CACHE[key]


def kernel(vector, matrix, input_lengths, W_vec, W_mat, w_attn):
    vector = np.asarray(vector, dtype=np.float32)
    matrix = np.ascontiguousarray(np.asarray(matrix, dtype=np.float32))
    lengths = np.asarray(input_lengths)
    W_vec = np.ascontiguousarray(np.asarray(W_vec, dtype=np.float32))
    W_mat = np.ascontiguousarray(np.asarray(W_mat, dtype=np.float32))
    w_attn = np.asarray(w_attn, dtype=np.float32)

    waT = w_attn.reshape(AT, 128).T
    ident = np.eye(128, dtype=np.float32)

    # sort batches by length, deal round-robin: core c slot j gets
    # sorted[j * N_CORES + c]; slot j's tile count covers its max length.
    # Shortest slot runs first (small first load), second-shortest last
    # (short drain tail).
    order = np.argsort(lengths, kind="stable")
    slot_of = order.reshape(BPC, N_CORES)  # [slot, core] -> original batch
    perm = [0] + list(range(2, BPC)) + [1]
    slot_of = slot_of[perm]
    lens_i = lengths.astype(np.int64)
    st_slots = []
    for j in range(BPC):
        mx = int(lens_i[slot_of[j]].max())
        st_slots.append(min(ST, max(2, -(-mx // 128))))
    nc = _get_program(st_slots)

    in_maps = []
    for c in range(N_CORES):
        idx = slot_of[:, c]  # original batch index per slot
        # packed consts: [ident | waT | vecT as [p, dt, b]]
        vecT = vector[idx].T.reshape(DT, 128, BPC).transpose(1, 0, 2)
        consts = np.concatenate(
            [ident, waT, vecT.reshape(128, DT * BPC)], axis=1)
        in_maps.append({
            "mat": np.ascontiguousarray(matrix[idx]),
            "wv": W_vec,
            "wm": W_mat,
            "consts": np.ascontiguousarray(consts),
            "lens": np.ascontiguousarray(
                lengths[idx].astype(np.float32).reshape(1, BPC)),
        })

    res = run_bass_kernel_spmd(nc, in_maps, core_ids=list(range(N_CORES)))

    reps = np.empty((B, DM), np.float32)
    attn = np.empty((B, S), np.float32)
    for c in range(N_CORES):
        reps[slot_of[:, c]] = res.results[c]["reps"]
        attn[slot_of[:, c]] = res.results[c]["attn"]
    kernel.last_results = res
    return reps, attn


# revision 38
# speedup vs baseline: 1.0462x; 1.0462x over previous
"""Trainium2 Bass kernel for AttentionPair pooling.

Computation (per batch row b):
    t1 = vector @ W_vec                        [B, A]
    t2 = matrix @ W_mat                        [B, S, A]
    t3 = relu(t1[:, None, :] + t2)             [B, S, A]
    logits = t3 @ w_attn                       [B, S]
    attn = masked softmax over valid s         [B, S]
    reps = einsum('bsd,bs->bd', matrix, attn)  [B, D]

Sharding: data-parallel over batch across 8 NeuronCores (8 batches per
core); the three weight tensors are replicated.

Length specialization: positions s >= input_lengths[b] contribute
nothing (attn is exactly 0 there), so per batch only
ceil(len / 128) s-tiles of work are needed.  Batches are sorted by
length and dealt round-robin to the 8 cores, so the shared SPMD
program's slot j only needs tiles for the max length in that slot.
The program is built (and cached) per slot-tile-count tuple at call
time from the actual input_lengths.

Per-core device program (fully unrolled, Tile framework):
  - matrix rows stream in natural [s, d] layout, one batch at a time
  - each batch tile is transposed on the PE (128x128 fp32r transposes)
    into [d, s] layout for the t2 contraction over d
  - t2^T[a, s] accumulates in PSUM over 8 d-tiles (fp32r matmuls)
  - PSUM evacuation is fused with the +t1 bias and relu on the Scalar
    engine; transpose evacuations alternate DVE/ACT
  - logits via w_attn as a [128, 1] stationary operand, per-row masked
    softmax on partition 0 reading logits straight from PSUM
  - attn-weighted sum (reps) is software-pipelined one batch behind so
    the in-order PE queue never stalls on a softmax chain

The global max subtraction in the reference cancels in the per-row
normalization, so a per-row max is used instead (mathematically
identical, numerically safe).
"""

import numpy as np

import concourse.tile as tile
from concourse import bacc, mybir
from concourse.bass_utils import run_bass_kernel_spmd

B, S, DV, DA, DM = 64, 512, 1024, 512, 1024
N_CORES = 8
BPC = B // N_CORES  # batches per core

F32 = mybir.dt.float32
MM_DT = mybir.dt.float32r  # PE runs fp32 data at bf16 rate in this mode

DT = DM // 128   # 8 d-tiles
AT = DA // 128   # 4 a-tiles
ST = S // 128    # 4 s-tiles (max)


def _r(ap):
    return ap.bitcast(MM_DT)


def build_program(st_slots):
    """st_slots[j] = number of 128-row s-tiles to process for batch slot j
    (identical across cores; each >= 2 so fp32r matmuls keep full rate)."""
    nc = bacc.Bacc("TRN2", target_bir_lowering=False, debug=False,
                   num_devices=N_CORES)

    # fp32r-typed DRAM inputs: the PE consumes them in fp32r mode, plain
    # HWDGE DMA (no SWDGE cast pass), bits are the raw fp32 values.
    mat_d = nc.dram_tensor("mat", [BPC, S, DM], MM_DT, kind="ExternalInput")
    wv_d = nc.dram_tensor("wv", [DV, DA], MM_DT, kind="ExternalInput")
    wm_d = nc.dram_tensor("wm", [DM, DA], MM_DT, kind="ExternalInput")
    # packed [ident(128) | waT(AT) | vecT(DT*BPC)] columns, one DMA
    CW = 128 + AT + DT * BPC
    consts_d = nc.dram_tensor("consts", [128, CW], MM_DT, kind="ExternalInput")
    lens_d = nc.dram_tensor("lens", [1, BPC], F32, kind="ExternalInput")

    reps_d = nc.dram_tensor("reps", [BPC, DM], F32, kind="ExternalOutput")
    attn_d = nc.dram_tensor("attn", [BPC, S], F32, kind="ExternalOutput")

    with tile.TileContext(nc) as tc:
        with (
            tc.tile_pool(name="const", bufs=1) as const,
            tc.tile_pool(name="mat_nat", bufs=4) as mat_pool,
            tc.tile_pool(name="matT", bufs=2) as matT_pool,
            tc.tile_pool(name="t3", bufs=2) as t3_pool,
            tc.tile_pool(name="rows", bufs=2) as rows,
            tc.tile_pool(name="ps_tr", bufs=2, space="PSUM") as ps_tr,
            tc.tile_pool(name="ps_mm", bufs=2, space="PSUM") as ps_mm,
            tc.tile_pool(name="ps_lg", bufs=1, space="PSUM") as ps_lg,
            tc.tile_pool(name="ps_at", bufs=1, space="PSUM") as ps_at,
            tc.tile_pool(name="ps_rep", bufs=2, space="PSUM") as ps_rep,
        ):
            # ---- matrix loads + packed constants on the sync ring (no
            # ---- data-dependent waits ever sit ahead of these triggers);
            # ---- weights on the scalar ring run concurrently; outputs go
            # ---- on the scalar ring so their waits can't block loads.
            # ---- The first (shortest) slot's matrix is triggered first so
            # ---- the PE starts as early as possible.
            mat_tiles = {}

            def load_mat(j):
                t = mat_pool.tile([128, ST, DM], MM_DT, tag="mat_nat")
                stj = st_slots[j]
                nc.sync.dma_start(
                    t[:, 0:stj, :],
                    mat_d.ap()[j:j + 1, 0:128 * stj, :].rearrange(
                        "o (t p) d -> p (o t) d", p=128),
                )
                mat_tiles[j] = t

            load_mat(0)

            consts_sb = const.tile([128, CW], MM_DT)
            nc.sync.dma_start(consts_sb[:], consts_d.ap()[:])
            ident = consts_sb[:, 0:128]
            waT_sb = consts_sb[:, 128:128 + AT]
            vecT_sb = consts_sb[:, 128 + AT:].rearrange("p (t b) -> p t b", b=BPC)
            lens_sb = const.tile([1, BPC], F32)
            nc.sync.dma_start(lens_sb[:], lens_d.ap()[:])

            wv_sb = const.tile([128, DT, DA], MM_DT)
            nc.scalar.dma_start(wv_sb[:], wv_d.ap().rearrange("(t p) a -> p t a", p=128))
            wm_sb = const.tile([128, DT, DA], MM_DT)
            for h in range(2):
                nc.scalar.dma_start(
                    wm_sb[:, 4 * h:4 * (h + 1), :],
                    wm_d.ap()[512 * h:512 * (h + 1), :].rearrange(
                        "(t p) a -> p t a", p=128))

            iota_i = const.tile([1, S], mybir.dt.int32)
            nc.gpsimd.iota(iota_i[:], pattern=[[1, S]], base=0, channel_multiplier=0)
            iota_f = const.tile([1, S], F32)
            nc.vector.tensor_copy(iota_f[:], iota_i[:])

            t1T_sb = const.tile([128, AT, BPC], F32)

            def emit_t1():
                # t1 = vector @ W_vec: wide matmuls into [b, a] layout, then
                # PE-transpose the [8, 512] result into per-partition [a, b]
                ps_t1 = ps_lg.tile([BPC, DA], F32, tag="ps_lg")
                for dt in range(DT):
                    nc.tensor.matmul(
                        ps_t1[:],
                        vecT_sb[:, dt, :],
                        wv_sb[:, dt, :],
                        start=(dt == 0), stop=(dt == DT - 1),
                    )
                t1_ba = const.tile([BPC, DA], MM_DT)
                nc.vector.tensor_copy(t1_ba[:], _r(ps_t1[:]))
                ps_tt = ps_tr.tile([128, AT, BPC], F32, tag="ps_tr")
                for at in range(AT):
                    nc.tensor.matmul(
                        _r(ps_tt[:, at, :]),
                        t1_ba[:, at * 128:(at + 1) * 128],
                        ident[0:BPC, 0:BPC],
                        is_transpose=True,
                    )
                nc.vector.tensor_copy(t1T_sb[:], ps_tt[:])

            def emit_reps(attn_row, mat_nat, b, stb):
                # attn^T as [s % 128, s // 128] columns via tiny PE
                # transposes.  fp32r matmuls need even element counts and
                # 8B-aligned PSUM destinations, so transpose two rows at a
                # time (row 1 is junk and ignored downstream).
                ps_a = ps_at.tile([128, 2 * ST], F32, tag="ps_at")
                for st in range(stb):
                    nc.tensor.matmul(
                        _r(ps_a[:, 2 * st:2 * st + 2]),
                        attn_row[0:2, st * 128:(st + 1) * 128],
                        ident[0:2, 0:2],
                        is_transpose=True,
                    )
                attnT = rows.tile([128, 2 * ST], MM_DT)
                nc.vector.tensor_copy(attnT[:, 0:2 * stb], _r(ps_a[:, 0:2 * stb]))

                # reps[d] = sum_s attn[s] * mat[s, d]
                reps_row = rows.tile([1, DM], F32)
                for nh in range(2):
                    ps_r = ps_rep.tile([1, DA], F32, tag="ps_rep")
                    for st in range(stb):
                        nc.tensor.matmul(
                            ps_r[:],
                            attnT[:, 2 * st:2 * st + 1],
                            mat_nat[:, st, nh * DA:(nh + 1) * DA],
                            start=(st == 0), stop=(st == stb - 1),
                        )
                    nc.scalar.copy(reps_row[0:1, nh * DA:(nh + 1) * DA], ps_r[:])
                nc.scalar.dma_start(reps_d.ap()[b:b + 1, :], reps_row[0:1, :])

            def emit_logits_softmax(t3_sb, b, stb):
                wb = 128 * stb
                # logits[s] = sum_a t3[a, s] * w_attn[a]
                ps_l = ps_lg.tile([1, S], F32, tag="ps_lg")
                for at in range(AT):
                    nc.tensor.matmul(
                        ps_l[:, 0:wb],
                        waT_sb[:, at:at + 1],
                        t3_sb[:, at, 0:wb],
                        start=(at == 0), stop=(at == AT - 1),
                    )
                return ps_l

            def emit_softmax(ps_l, b, stb):
                wb = 128 * stb
                # masked softmax on partition 0, reading logits from PSUM
                neg_mx = rows.tile([1, 1], F32)
                nc.vector.tensor_reduce(
                    out=neg_mx[:], in_=ps_l[0:1, 0:wb],
                    op=mybir.AluOpType.max, axis=mybir.AxisListType.X,
                    negate=True,
                )
                exp_row = rows.tile([1, S], F32)
                nc.scalar.activation(
                    exp_row[0:1, 0:wb], ps_l[0:1, 0:wb],
                    mybir.ActivationFunctionType.Exp,
                    bias=neg_mx[0:1, 0:1],
                )
                mask_row = rows.tile([1, S], F32)
                nc.vector.tensor_scalar(
                    out=mask_row[0:1, 0:wb], in0=iota_f[0:1, 0:wb],
                    scalar1=lens_sb[0:1, b:b + 1], scalar2=None,
                    op0=mybir.AluOpType.is_lt,
                )
                masked = rows.tile([1, S], F32)
                nc.vector.tensor_tensor(
                    out=masked[0:1, 0:wb], in0=exp_row[0:1, 0:wb],
                    in1=mask_row[0:1, 0:wb],
                    op=mybir.AluOpType.mult,
                )
                ssum = rows.tile([1, 1], F32)
                nc.vector.tensor_reduce(
                    out=ssum[:], in_=masked[0:1, 0:wb],
                    op=mybir.AluOpType.add, axis=mybir.AxisListType.X,
                )
                rcp = rows.tile([1, 1], F32)
                nc.vector.reciprocal(rcp[:], ssum[:])
                attn_row = rows.tile([2, S], MM_DT)
                nc.gpsimd.memset(attn_row[0:2, :].bitcast(F32), 0.0)
                nc.vector.tensor_scalar_mul(
                    attn_row[0:1, 0:wb], masked[0:1, 0:wb], rcp[0:1, 0:1])
                nc.scalar.dma_start(attn_d.ap()[b:b + 1, :], attn_row[0:1, :].bitcast(F32))
                return attn_row

            # Two-stage software pipeline: logits+softmax for batch b-1 and
            # the attn-weighted sum for batch b-2 are emitted while batch
            # b's transposes/t2 keep the PE dense, so the in-order PE queue
            # never waits on the relu or softmax chains.
            pend_sm = None    # (t3_sb, b, stb) awaiting logits + softmax
            pend_reps = None  # (attn_row, mat_nat, b, stb)

            for b in range(BPC):
                stb = st_slots[b]
                wb = 128 * stb  # valid s width this slot

                if b + 1 < BPC:
                    load_mat(b + 1)
                mat_nat = mat_tiles.pop(b)

                # transpose to [d % 128, d // 128, s] via PE; evacuations
                # alternate between DVE and ACT so neither engine gates
                matT = matT_pool.tile([128, DT, S], MM_DT)
                for dt in range(DT):
                    ps = ps_tr.tile([128, S], F32, tag="ps_tr")
                    for st in range(stb):
                        nc.tensor.matmul(
                            _r(ps[:, st * 128:(st + 1) * 128]),
                            mat_nat[:, st, dt * 128:(dt + 1) * 128],
                            ident[:],
                            is_transpose=True,
                        )
                    if dt % 2 == 0:
                        nc.vector.tensor_copy(matT[:, dt, 0:wb], _r(ps[:, 0:wb]))
                    else:
                        nc.scalar.copy(matT[:, dt, 0:wb], _r(ps[:, 0:wb]))

                # t2^T[a, s] + t1 bias + relu -> t3 [a % 128, a // 128, s]
                t3_sb = t3_pool.tile([128, AT, S], MM_DT)

                def t2_mms(at):
                    ps = ps_mm.tile([128, S], F32, tag="ps_mm")
                    for dt in range(DT):
                        nc.tensor.matmul(
                            ps[:, 0:wb],
                            wm_sb[:, dt, at * 128:(at + 1) * 128],
                            matT[:, dt, 0:wb],
                            start=(dt == 0), stop=(dt == DT - 1),
                        )
                    return ps

                def t2_relu(at, ps):
                    nc.scalar.activation(
                        t3_sb[:, at, 0:wb], ps[:, 0:wb],
                        mybir.ActivationFunctionType.Relu,
                        bias=t1T_sb[:, at, b:b + 1],
                    )

                if b == 0:
                    emit_t1()
                for at in range(AT):
                    t2_relu(at, t2_mms(at))

                if pend_sm is not None:
                    p_t3, p_b, p_stb, p_mat = pend_sm
                    ps_l = emit_logits_softmax(p_t3, p_b, p_stb)
                    if pend_reps is not None:
                        emit_reps(*pend_reps)
                    attn_row = emit_softmax(ps_l, p_b, p_stb)
                    pend_reps = (attn_row, p_mat, p_b, p_stb)
                pend_sm = (t3_sb, b, stb, mat_nat)

            # drain the pipeline
            p_t3, p_b, p_stb, p_mat = pend_sm
            ps_l = emit_logits_softmax(p_t3, p_b, p_stb)
            if pend_reps is not None:
                emit_reps(*pend_reps)
            attn_row = emit_softmax(ps_l, p_b, p_stb)
            emit_reps(attn_row, p_mat, p_b, p_stb)

    nc.compile()
    return nc


_PROGRAM_CACHE = {}


def _get_program(st_slots):
    key = tuple(st_slots)
    if key not in _PROGRAM_CACHE:
        _PROGRAM_CACHE[key] = build_program(key)
    return _PROGRAM_CACHE[key]


def kernel(vector, matrix, input_lengths, W_vec, W_mat, w_attn):
    vector = np.asarray(vector, dtype=np.float32)
    matrix = np.ascontiguousarray(np.asarray(matrix, dtype=np.float32))
    lengths = np.asarray(input_lengths)
    W_vec = np.ascontiguousarray(np.asarray(W_vec, dtype=np.float32))
    W_mat = np.ascontiguousarray(np.asarray(W_mat, dtype=np.float32))
    w_attn = np.asarray(w_attn, dtype=np.float32)

    waT = w_attn.reshape(AT, 128).T
    ident = np.eye(128, dtype=np.float32)

    # sort batches by length, deal round-robin: core c slot j gets
    # sorted[j * N_CORES + c]; slot j's tile count covers its max length.
    # Shortest slot runs first (small first load), second-shortest last
    # (short drain tail).
    order = np.argsort(lengths, kind="stable")
    slot_of = order.reshape(BPC, N_CORES)  # [slot, core] -> original batch
    perm = [0] + list(range(2, BPC)) + [1]
    slot_of = slot_of[perm]
    lens_i = lengths.astype(np.int64)
    st_slots = []
    for j in range(BPC):
        mx = int(lens_i[slot_of[j]].max())
        st_slots.append(min(ST, max(2, -(-mx // 128))))
    nc = _get_program(st_slots)

    in_maps = []
    for c in range(N_CORES):
        idx = slot_of[:, c]  # original batch index per slot
        # packed consts: [ident | waT | vecT as [p, dt, b]]
        vecT = vector[idx].T.reshape(DT, 128, BPC).transpose(1, 0, 2)
        consts = np.concatenate(
            [ident, waT, vecT.reshape(128, DT * BPC)], axis=1)
        in_maps.append({
            "mat": np.ascontiguousarray(matrix[idx]),
            "wv": W_vec,
            "wm": W_mat,
            "consts": np.ascontiguousarray(consts),
            "lens": np.ascontiguousarray(
                lengths[idx].astype(np.float32).reshape(1, BPC)),
        })

    res = run_bass_kernel_spmd(nc, in_maps, core_ids=list(range(N_CORES)))

    reps = np.empty((B, DM), np.float32)
    attn = np.empty((B, S), np.float32)
    for c in range(N_CORES):
        reps[slot_of[:, c]] = res.results[c]["reps"]
        attn[slot_of[:, c]] = res.results[c]["attn"]
    kernel.last_results = res
    return reps, attn


# revision 39
# speedup vs baseline: 1.0825x; 1.0347x over previous
"""Trainium2 Bass kernel for AttentionPair pooling.

Computation (per batch row b):
    t1 = vector @ W_vec                        [B, A]
    t2 = matrix @ W_mat                        [B, S, A]
    t3 = relu(t1[:, None, :] + t2)             [B, S, A]
    logits = t3 @ w_attn                       [B, S]
    attn = masked softmax over valid s         [B, S]
    reps = einsum('bsd,bs->bd', matrix, attn)  [B, D]

Sharding: data-parallel over batch across 8 NeuronCores (8 batches per
core); the three weight tensors are replicated.

Length specialization: positions s >= input_lengths[b] contribute
nothing (attn is exactly 0 there), so per batch only
ceil(len / 128) s-tiles of work are needed.  Batches are sorted by
length and dealt round-robin to the 8 cores, so the shared SPMD
program's slot j only needs tiles for the max length in that slot.
The program is built (and cached) per slot-tile-count tuple at call
time from the actual input_lengths.

Per-core device program (fully unrolled, Tile framework):
  - matrix rows stream in natural [s, d] layout, one batch at a time
  - each batch tile is transposed on the PE (128x128 fp32r transposes)
    into [d, s] layout for the t2 contraction over d
  - t2^T[a, s] accumulates in PSUM over 8 d-tiles (fp32r matmuls)
  - PSUM evacuation is fused with the +t1 bias and relu on the Scalar
    engine; transpose evacuations alternate DVE/ACT
  - logits via w_attn as a [128, 1] stationary operand, per-row masked
    softmax on partition 0 reading logits straight from PSUM
  - attn-weighted sum (reps) is software-pipelined one batch behind so
    the in-order PE queue never stalls on a softmax chain

The global max subtraction in the reference cancels in the per-row
normalization, so a per-row max is used instead (mathematically
identical, numerically safe).
"""

import numpy as np

import concourse.tile as tile
from concourse import bacc, mybir
from concourse.bass_utils import run_bass_kernel_spmd

B, S, DV, DA, DM = 64, 512, 1024, 512, 1024
N_CORES = 8
BPC = B // N_CORES  # batches per core

F32 = mybir.dt.float32
MM_DT = mybir.dt.float32r  # PE runs fp32 data at bf16 rate in this mode

DT = DM // 128   # 8 d-tiles
AT = DA // 128   # 4 a-tiles
ST = S // 128    # 4 s-tiles (max)


def _r(ap):
    return ap.bitcast(MM_DT)


def build_program(st_slots):
    """st_slots[j] = number of 128-row s-tiles to process for batch slot j
    (identical across cores; each >= 2 so fp32r matmuls keep full rate)."""
    nc = bacc.Bacc("TRN2", target_bir_lowering=False, debug=False,
                   num_devices=N_CORES)

    # fp32r-typed DRAM inputs: the PE consumes them in fp32r mode, plain
    # HWDGE DMA (no SWDGE cast pass), bits are the raw fp32 values.
    mat_d = nc.dram_tensor("mat", [BPC, S, DM], MM_DT, kind="ExternalInput")
    wv_d = nc.dram_tensor("wv", [DV, DA], MM_DT, kind="ExternalInput")
    wm_d = nc.dram_tensor("wm", [DM, DA], MM_DT, kind="ExternalInput")
    # packed [ident(128) | waT(AT) | vecT(DT*BPC)] columns, one DMA
    CW = 128 + AT + DT * BPC
    consts_d = nc.dram_tensor("consts", [128, CW], MM_DT, kind="ExternalInput")
    lens_d = nc.dram_tensor("lens", [1, BPC], F32, kind="ExternalInput")

    reps_d = nc.dram_tensor("reps", [BPC, DM], F32, kind="ExternalOutput")
    attn_d = nc.dram_tensor("attn", [BPC, S], F32, kind="ExternalOutput")

    with tile.TileContext(nc) as tc:
        with (
            tc.tile_pool(name="const", bufs=1) as const,
            tc.tile_pool(name="mat_nat", bufs=4) as mat_pool,
            tc.tile_pool(name="matT", bufs=2) as matT_pool,
            tc.tile_pool(name="t3", bufs=2) as t3_pool,
            tc.tile_pool(name="rows", bufs=2) as rows,
            tc.tile_pool(name="ps_tr", bufs=2, space="PSUM") as ps_tr,
            tc.tile_pool(name="ps_mm", bufs=2, space="PSUM") as ps_mm,
            tc.tile_pool(name="ps_lg", bufs=1, space="PSUM") as ps_lg,
            tc.tile_pool(name="ps_at", bufs=1, space="PSUM") as ps_at,
            tc.tile_pool(name="ps_rep", bufs=2, space="PSUM") as ps_rep,
        ):
            # ---- matrix loads + packed constants on the sync ring (no
            # ---- data-dependent waits ever sit ahead of these triggers);
            # ---- weights on the scalar ring run concurrently; outputs go
            # ---- on the scalar ring so their waits can't block loads.
            # ---- The first (shortest) slot's matrix is triggered first so
            # ---- the PE starts as early as possible.
            mat_tiles = {}

            def load_mat(j):
                t = mat_pool.tile([128, ST, DM], MM_DT, tag="mat_nat")
                stj = st_slots[j]
                nc.sync.dma_start(
                    t[:, 0:stj, :],
                    mat_d.ap()[j:j + 1, 0:128 * stj, :].rearrange(
                        "o (t p) d -> p (o t) d", p=128),
                )
                mat_tiles[j] = t

            load_mat(0)

            consts_sb = const.tile([128, CW], MM_DT)
            nc.sync.dma_start(consts_sb[:], consts_d.ap()[:])
            ident = consts_sb[:, 0:128]
            waT_sb = consts_sb[:, 128:128 + AT]
            vecT_sb = consts_sb[:, 128 + AT:].rearrange("p (t b) -> p t b", b=BPC)
            lens_sb = const.tile([1, BPC], F32)
            nc.sync.dma_start(lens_sb[:], lens_d.ap()[:])

            wv_sb = const.tile([128, DT, DA], MM_DT)
            nc.scalar.dma_start(wv_sb[:], wv_d.ap().rearrange("(t p) a -> p t a", p=128))
            wm_sb = const.tile([128, DT, DA], MM_DT)
            for h in range(2):
                nc.scalar.dma_start(
                    wm_sb[:, 4 * h:4 * (h + 1), :],
                    wm_d.ap()[512 * h:512 * (h + 1), :].rearrange(
                        "(t p) a -> p t a", p=128))

            iota_i = const.tile([1, S], mybir.dt.int32)
            nc.gpsimd.iota(iota_i[:], pattern=[[1, S]], base=0, channel_multiplier=0)
            iota_f = const.tile([1, S], F32)
            nc.vector.tensor_copy(iota_f[:], iota_i[:])

            t1T_sb = const.tile([128, AT, BPC], F32)

            def emit_t1():
                # t1 = vector @ W_vec: wide matmuls into [b, a] layout, then
                # PE-transpose the [8, 512] result into per-partition [a, b]
                ps_t1 = ps_lg.tile([BPC, DA], F32, tag="ps_lg")
                for dt in range(DT):
                    nc.tensor.matmul(
                        ps_t1[:],
                        vecT_sb[:, dt, :],
                        wv_sb[:, dt, :],
                        start=(dt == 0), stop=(dt == DT - 1),
                    )
                t1_ba = const.tile([BPC, DA], MM_DT)
                nc.vector.tensor_copy(t1_ba[:], _r(ps_t1[:]))
                ps_tt = ps_tr.tile([128, AT, BPC], F32, tag="ps_tr")
                for at in range(AT):
                    nc.tensor.matmul(
                        _r(ps_tt[:, at, :]),
                        t1_ba[:, at * 128:(at + 1) * 128],
                        ident[0:BPC, 0:BPC],
                        is_transpose=True,
                    )
                nc.vector.tensor_copy(t1T_sb[:], ps_tt[:])

            def emit_reps(attn_row, mat_nat, b, stb):
                # attn^T as [s % 128, s // 128] columns via tiny PE
                # transposes.  fp32r matmuls need even element counts and
                # 8B-aligned PSUM destinations, so transpose two rows at a
                # time (row 1 is junk and ignored downstream).
                ps_a = ps_at.tile([128, 2 * ST], F32, tag="ps_at")
                for st in range(stb):
                    nc.tensor.matmul(
                        _r(ps_a[:, 2 * st:2 * st + 2]),
                        attn_row[0:2, st * 128:(st + 1) * 128],
                        ident[0:2, 0:2],
                        is_transpose=True,
                    )
                attnT = rows.tile([128, 2 * ST], MM_DT)
                nc.vector.tensor_copy(attnT[:, 0:2 * stb], _r(ps_a[:, 0:2 * stb]))

                # reps[d] = sum_s attn[s] * mat[s, d]
                reps_row = rows.tile([1, DM], F32)
                for nh in range(2):
                    ps_r = ps_rep.tile([1, DA], F32, tag="ps_rep")
                    for st in range(stb):
                        nc.tensor.matmul(
                            ps_r[:],
                            attnT[:, 2 * st:2 * st + 1],
                            mat_nat[:, st, nh * DA:(nh + 1) * DA],
                            start=(st == 0), stop=(st == stb - 1),
                        )
                    nc.scalar.copy(reps_row[0:1, nh * DA:(nh + 1) * DA], ps_r[:])
                nc.scalar.dma_start(reps_d.ap()[b:b + 1, :], reps_row[0:1, :])

            def emit_logits_softmax(t3_sb, b, stb):
                wb = 128 * stb
                # logits[s] = sum_a t3[a, s] * w_attn[a]
                ps_l = ps_lg.tile([1, S], F32, tag="ps_lg")
                for at in range(AT):
                    nc.tensor.matmul(
                        ps_l[:, 0:wb],
                        waT_sb[:, at:at + 1],
                        t3_sb[:, at, 0:wb],
                        start=(at == 0), stop=(at == AT - 1),
                    )
                return ps_l

            def emit_softmax(ps_l, b, stb):
                wb = 128 * stb
                # masked softmax on partition 0, reading logits from PSUM
                neg_mx = rows.tile([1, 1], F32)
                nc.vector.tensor_reduce(
                    out=neg_mx[:], in_=ps_l[0:1, 0:wb],
                    op=mybir.AluOpType.max, axis=mybir.AxisListType.X,
                    negate=True,
                )
                exp_row = rows.tile([1, S], F32)
                nc.scalar.activation(
                    exp_row[0:1, 0:wb], ps_l[0:1, 0:wb],
                    mybir.ActivationFunctionType.Exp,
                    bias=neg_mx[0:1, 0:1],
                )
                mask_row = rows.tile([1, S], F32)
                nc.vector.tensor_scalar(
                    out=mask_row[0:1, 0:wb], in0=iota_f[0:1, 0:wb],
                    scalar1=lens_sb[0:1, b:b + 1], scalar2=None,
                    op0=mybir.AluOpType.is_lt,
                )
                masked = rows.tile([1, S], F32)
                nc.vector.tensor_tensor(
                    out=masked[0:1, 0:wb], in0=exp_row[0:1, 0:wb],
                    in1=mask_row[0:1, 0:wb],
                    op=mybir.AluOpType.mult,
                )
                ssum = rows.tile([1, 1], F32)
                nc.vector.tensor_reduce(
                    out=ssum[:], in_=masked[0:1, 0:wb],
                    op=mybir.AluOpType.add, axis=mybir.AxisListType.X,
                )
                rcp = rows.tile([1, 1], F32)
                nc.vector.reciprocal(rcp[:], ssum[:])
                attn_row = rows.tile([2, S], MM_DT)
                nc.gpsimd.memset(attn_row[0:2, :].bitcast(F32), 0.0)
                nc.vector.tensor_scalar_mul(
                    attn_row[0:1, 0:wb], masked[0:1, 0:wb], rcp[0:1, 0:1])
                nc.scalar.dma_start(attn_d.ap()[b:b + 1, :], attn_row[0:1, :].bitcast(F32))
                return attn_row

            # Two-stage software pipeline: logits+softmax for batch b-1 and
            # the attn-weighted sum for batch b-2 are emitted while batch
            # b's transposes/t2 keep the PE dense, so the in-order PE queue
            # never waits on the relu or softmax chains.
            pend_sm = None    # (t3_sb, b, stb) awaiting logits + softmax
            pend_reps = None  # (attn_row, mat_nat, b, stb)

            for b in range(BPC):
                stb = st_slots[b]
                wb = 128 * stb  # valid s width this slot

                if b + 1 < BPC:
                    load_mat(b + 1)
                mat_nat = mat_tiles.pop(b)

                # transpose to [d % 128, d // 128, s] via PE; each group's
                # evacuation is split across DVE and ACT so the PSUM slot
                # frees in half the time and the PE never waits on a slot
                matT = matT_pool.tile([128, DT, S], MM_DT)
                half = (wb // 256) * 128  # even fp32r element counts
                for dt in range(DT):
                    ps = ps_tr.tile([128, S], F32, tag="ps_tr")
                    for st in range(stb):
                        nc.tensor.matmul(
                            _r(ps[:, st * 128:(st + 1) * 128]),
                            mat_nat[:, st, dt * 128:(dt + 1) * 128],
                            ident[:],
                            is_transpose=True,
                        )
                    nc.vector.tensor_copy(matT[:, dt, 0:half], _r(ps[:, 0:half]))
                    nc.scalar.copy(matT[:, dt, half:wb], _r(ps[:, half:wb]))

                # t2^T[a, s] + t1 bias + relu -> t3 [a % 128, a // 128, s]
                t3_sb = t3_pool.tile([128, AT, S], MM_DT)

                def t2_mms(at):
                    ps = ps_mm.tile([128, S], F32, tag="ps_mm")
                    for dt in range(DT):
                        nc.tensor.matmul(
                            ps[:, 0:wb],
                            wm_sb[:, dt, at * 128:(at + 1) * 128],
                            matT[:, dt, 0:wb],
                            start=(dt == 0), stop=(dt == DT - 1),
                        )
                    return ps

                def t2_relu(at, ps):
                    nc.scalar.activation(
                        t3_sb[:, at, 0:wb], ps[:, 0:wb],
                        mybir.ActivationFunctionType.Relu,
                        bias=t1T_sb[:, at, b:b + 1],
                    )

                if b == 0:
                    emit_t1()
                for at in range(AT):
                    t2_relu(at, t2_mms(at))

                if pend_sm is not None:
                    p_t3, p_b, p_stb, p_mat = pend_sm
                    ps_l = emit_logits_softmax(p_t3, p_b, p_stb)
                    if pend_reps is not None:
                        emit_reps(*pend_reps)
                    attn_row = emit_softmax(ps_l, p_b, p_stb)
                    pend_reps = (attn_row, p_mat, p_b, p_stb)
                pend_sm = (t3_sb, b, stb, mat_nat)

            # drain the pipeline
            p_t3, p_b, p_stb, p_mat = pend_sm
            ps_l = emit_logits_softmax(p_t3, p_b, p_stb)
            if pend_reps is not None:
                emit_reps(*pend_reps)
            attn_row = emit_softmax(ps_l, p_b, p_stb)
            emit_reps(attn_row, p_mat, p_b, p_stb)

    nc.compile()
    return nc


_PROGRAM_CACHE = {}


def _get_program(st_slots):
    key = tuple(st_slots)
    if key not in _PROGRAM_CACHE:
        _PROGRAM_CACHE[key] = build_program(key)
    return _PROGRAM_CACHE[key]


def kernel(vector, matrix, input_lengths, W_vec, W_mat, w_attn):
    vector = np.asarray(vector, dtype=np.float32)
    matrix = np.ascontiguousarray(np.asarray(matrix, dtype=np.float32))
    lengths = np.asarray(input_lengths)
    W_vec = np.ascontiguousarray(np.asarray(W_vec, dtype=np.float32))
    W_mat = np.ascontiguousarray(np.asarray(W_mat, dtype=np.float32))
    w_attn = np.asarray(w_attn, dtype=np.float32)

    waT = w_attn.reshape(AT, 128).T
    ident = np.eye(128, dtype=np.float32)

    # sort batches by length, deal round-robin: core c slot j gets
    # sorted[j * N_CORES + c]; slot j's tile count covers its max length.
    # Shortest slot runs first (small first load), second-shortest last
    # (short drain tail).
    order = np.argsort(lengths, kind="stable")
    slot_of = order.reshape(BPC, N_CORES)  # [slot, core] -> original batch
    perm = [0] + list(range(2, BPC)) + [1]
    slot_of = slot_of[perm]
    lens_i = lengths.astype(np.int64)
    st_slots = []
    for j in range(BPC):
        mx = int(lens_i[slot_of[j]].max())
        st_slots.append(min(ST, max(2, -(-mx // 128))))
    nc = _get_program(st_slots)

    in_maps = []
    for c in range(N_CORES):
        idx = slot_of[:, c]  # original batch index per slot
        # packed consts: [ident | waT | vecT as [p, dt, b]]
        vecT = vector[idx].T.reshape(DT, 128, BPC).transpose(1, 0, 2)
        consts = np.concatenate(
            [ident, waT, vecT.reshape(128, DT * BPC)], axis=1)
        in_maps.append({
            "mat": np.ascontiguousarray(matrix[idx]),
            "wv": W_vec,
            "wm": W_mat,
            "consts": np.ascontiguousarray(consts),
            "lens": np.ascontiguousarray(
                lengths[idx].astype(np.float32).reshape(1, BPC)),
        })

    res = run_bass_kernel_spmd(nc, in_maps, core_ids=list(range(N_CORES)))

    reps = np.empty((B, DM), np.float32)
    attn = np.empty((B, S), np.float32)
    for c in range(N_CORES):
        reps[slot_of[:, c]] = res.results[c]["reps"]
        attn[slot_of[:, c]] = res.results[c]["attn"]
    kernel.last_results = res
    return reps, attn
